# revision 64
# baseline (speedup 1.0000x reference)
"""Multi-head differential attention on 8 Trainium2 NeuronCores.

Sharding: data-parallel over batch (B=2) x tensor-parallel over heads
(16 heads -> 4 per core). Core c handles batch c//4 and heads
4*(c%4) .. 4*(c%4)+3. Each core computes its heads' attention output and a
partial output projection; the host sums the 4 partials per batch.

V1 vs baseline: all matmul operands bf16 (enables PE fast-weight-load),
rope fused into the projection phase (no serialized vector phase), softmax
denominator accumulated on DVE instead of per-chunk PE ones-matmuls, causal
mask applied as a DVE multiply on E, and attention restructured qb-major so
RMS + output projection overlap with the next q-block's attention.
"""

import math
import os
import sys

sys.path.insert(0, "/opt/trn_rl_repo")

import numpy as np

B, S, HID, NH = 2, 2048, 2048, 16
HD = HID // NH          # 128
QKD = HD // 2           # 64
NCORES = 8
GRPS = NCORES // B      # head groups per batch
HPC = NH // GRPS        # heads per core = 4
LAYER_ID = 1
LAMBDA_INIT = 0.8 - 0.6 * math.exp(-0.3 * LAYER_ID)
EPS = 1e-6

NB = S // 512           # 4 seq blocks of 512
NKC = S // 128          # 16 key chunks of 128

_PROGRAM = None         # compiled bass program, reused across calls


def _build_program():
    import concourse.bass as bass
    import concourse.tile as tile
    from concourse import bacc, mybir

    f32 = mybir.dt.float32
    f32r = mybir.dt.float32r
    bf16 = mybir.dt.bfloat16
    Alu = mybir.AluOpType
    Act = mybir.ActivationFunctionType

    nc = bacc.Bacc(None, target_bir_lowering=False, debug=False)

    def din(name, shape, dt=bf16):
        return nc.dram_tensor(name, shape, dt, kind="ExternalInput").ap()

    io = {
        "xq_t": din("xq_t", [HID, S]),
        "xk_t": din("xk_t", [HID, S]),
        "xv_t": din("xv_t", [HID, S]),
        "wq_t": din("wq_t", [HID, 512]),
        "wk_t": din("wk_t", [HID, 512]),
        "wv_t": din("wv_t", [HID, 512]),
        "wo_t": din("wo_t", [512, HID]),
        "crep": din("crep", [128, S]),
        "srep": din("srep", [128, S]),
        "pmat": din("pmat", [128, 128]),
        "m01d": din("m01d", [128, 256]),
        "ones16": din("ones16", [128, 128]),
        "ones_a": din("ones_a", [128, 128], f32r),
        "neglam": din("neglam", [128, 1], f32),
    }
    y_t = nc.dram_tensor("y_t", [HID, S], bf16, kind="ExternalOutput").ap()

    from contextlib import ExitStack

    with tile.TileContext(nc) as tc, ExitStack() as ctx:
        persist = ctx.enter_context(tc.tile_pool(name="persist", bufs=1))
        constp = ctx.enter_context(tc.tile_pool(name="constp", bufs=1))

        # constants
        crep = constp.tile([128, S], bf16, name="crep_sb", tag="crep")
        srep = constp.tile([128, S], bf16, name="srep_sb", tag="srep")
        pmat = constp.tile([128, 128], bf16, name="pmat_sb", tag="pmat")
        m01d = constp.tile([128, 256], bf16, name="m01d_sb", tag="m01d")
        ones16 = constp.tile([128, 128], bf16, name="ones16_sb", tag="ones16")
        ones_a = constp.tile([128, 128], f32r, name="ones_a_sb", tag="ones_a")
        neglam = constp.tile([128, 1], f32, name="neglam_sb", tag="neglam")
        epsb = constp.tile([128, 1], f32, name="epsb", tag="epsb")
        nc.vector.memset(epsb[:], EPS)
        m01d3 = m01d[:].rearrange("p (g q) -> p g q", g=2)

        def load_consts():
            for t, key in ((crep, "crep"), (srep, "srep"), (pmat, "pmat"),
                           (m01d, "m01d"), (ones16, "ones16"),
                           (ones_a, "ones_a"), (neglam, "neglam")):
                nc.sync.dma_start(out=t[:], in_=io[key][:])

        # persistent tensors: Q^T/K^T per (map g, head-pair hp): [128, S]
        #   tile t = 2*g + hp; partitions [64*a, 64*a+64) hold head 2*hp+a.
        QT = [persist.tile([128, S], bf16, name=f"qt{t}", tag=f"qt{t}")
              for t in range(4)]
        # K^T split per head with the other head's 64 rows zeroed, so score
        # matmuls run 128-contract (full PE stream rate) with packed QT rhs.
        KZ = [[persist.tile([128, S], bf16, name=f"kz{t}_{a}", tag=f"kz{t}_{a}")
               for a in range(2)] for t in range(4)]
        for t in range(4):
            for a in range(2):
                eng = nc.vector if t % 2 == 0 else nc.gpsimd
                eng.memset(KZ[t][a][64 - 64 * a:128 - 64 * a, :], 0.0)
        # V natural layout per 128-seq chunk: [128 seq, 4 heads * 128 feat]
        VH = [persist.tile([128, 512], bf16, name=f"vh{s}", tag=f"vh{s}")
              for s in range(NKC)]
        # combined attention output (post RMS), transposed: [feat, seq]
        U = [persist.tile([128, S], bf16, name=f"u{h}", tag=f"u{h}")
             for h in range(HPC)]
        # output projection weights (loaded after phase P is issued)
        WO = [persist.tile([128, S], bf16, name=f"wo{h}", tag=f"wo{h}")
              for h in range(HPC)]

        # ---------------- phase P: q/k/v projections + fused rope ----------
        with tc.tile_pool(name="wp", bufs=1) as wp, \
             tc.tile_pool(name="xp", bufs=8) as xp, \
             tc.tile_pool(name="pp", bufs=1, space="PSUM") as pp, \
             tc.tile_pool(name="rxp", bufs=2, space="PSUM") as rxp, \
             tc.tile_pool(name="rtp", bufs=2) as rtp:
            for wname, xname, mode in (("wq_t", "xq_t", 0), ("wk_t", "xk_t", 1),
                                       ("wv_t", "xv_t", 2)):
                wt = [wp.tile([128, 512], bf16, name=f"{wname}_{kc}",
                              tag=f"w{kc}") for kc in range(NKC)]
                for kc in range(NKC):
                    nc.sync.dma_start(out=wt[kc][:],
                                      in_=io[wname][kc * 128:(kc + 1) * 128, :])
                if mode == 0:
                    # big constants queued behind the first weights so the
                    # first matmuls start sooner
                    load_consts()
                xin = io[xname]
                for n in range(NB):
                    ps = [pp.tile([128, 512], f32, name=f"pp{t}_{mode}_{n}",
                                  tag=f"pp{t}") for t in range(4)]
                    for kc in range(NKC):
                        xck = xp.tile([128, 512], bf16, name=f"x_{mode}_{n}_{kc}",
                                      tag="x")
                        nc.sync.dma_start(
                            out=xck[:],
                            in_=xin[kc * 128:(kc + 1) * 128,
                                    n * 512:(n + 1) * 512])
                        for t in range(4):
                            if mode < 2:
                                lhsT = wt[kc][:, t * 128:(t + 1) * 128]
                                rhs = xck[:]
                            else:
                                lhsT = xck[:, t * 128:(t + 1) * 128]
                                rhs = wt[kc][:]
                            nc.tensor.matmul(ps[t][:], lhsT, rhs,
                                             start=(kc == 0), stop=(kc == NKC - 1))
                    for t in range(4):
                        if mode == 0:
                            dst = QT[t][:, n * 512:(n + 1) * 512]
                        elif mode == 1:
                            kblk = rtp.tile([128, 512], bf16,
                                            name=f"kb_{n}_{t}", tag="kb")
                            dst = kblk[:]
                        else:
                            dst = VH[n * 4 + t][:]
                        nc.scalar.copy(dst, ps[t][:])
                        if mode < 2:
                            # rope in place: dst = dst*cos + (P dst)*sin
                            px = rxp.tile([128, 512], f32,
                                          name=f"px_{mode}_{n}_{t}", tag="px")
                            nc.tensor.matmul(px[:], pmat[:], dst,
                                             start=True, stop=True)
                            tmp = rtp.tile([128, 512], bf16,
                                           name=f"rt_{mode}_{n}_{t}", tag="rt")
                            nc.vector.tensor_mul(
                                tmp[:], px[:], srep[:, n * 512:(n + 1) * 512])
                            nc.vector.tensor_mul(
                                dst, dst, crep[:, n * 512:(n + 1) * 512])
                            nc.vector.tensor_add(dst, dst, tmp[:])
                        if mode == 1:
                            # scatter the two heads into zero-padded K tiles
                            # (on DVE: directly follows the rope ops in-queue)
                            nc.vector.tensor_copy(
                                KZ[t][0][0:64, n * 512:(n + 1) * 512],
                                kblk[0:64, :])
                            nc.vector.tensor_copy(
                                KZ[t][1][64:128, n * 512:(n + 1) * 512],
                                kblk[64:128, :])

        # wo loads issued after phase P: DMA overlaps with early attention
        for h in range(HPC):
            nc.sync.dma_start(out=WO[h][:],
                              in_=io["wo_t"][h * 128:(h + 1) * 128, :])

        # -------- phase A: attention + RMS + outproj, qb-major ------------
        with tc.tile_pool(name="sp", bufs=2, space="PSUM") as sp, \
             tc.tile_pool(name="pvp", bufs=1, space="PSUM") as pvp, \
             tc.tile_pool(name="pyp", bufs=2, space="PSUM") as pyp, \
             tc.tile_pool(name="ep", bufs=6) as ep, \
             tc.tile_pool(name="esp", bufs=2) as esp, \
             tc.tile_pool(name="cb", bufs=2) as cb, \
             tc.tile_pool(name="ys", bufs=4) as ys:

            def rms_outproj(qb):
                # RMS batch (sqrt batched: one act-table swap per qb) then
                # output projection; emitted one h-iteration into the next
                # q block so its engine waits overlap with attention.
                sds = []
                for h in range(HPC):
                    ublk = U[h][:, qb * 512:(qb + 1) * 512]
                    sq = cb.tile([128, 512], bf16, name=f"sq_{h}_{qb}", tag="sq")
                    nc.gpsimd.tensor_mul(sq[:], ublk, ublk)
                    ssq = pyp.tile([128, 512], f32, name=f"ssq_{h}_{qb}",
                                   tag="py")
                    nc.tensor.matmul(ssq[:], ones16[:], sq[:],
                                     start=True, stop=True)
                    sd = cb.tile([128, 512], f32, name=f"sd_{h}_{qb}",
                                 tag=f"sd{h}")
                    nc.scalar.activation(sd[:], ssq[:], Act.Sqrt,
                                         scale=1.0 / HD, bias=epsb[:])
                    sds.append(sd)
                for h in range(HPC):
                    ublk = U[h][:, qb * 512:(qb + 1) * 512]
                    rstd = cb.tile([128, 512], f32, name=f"rstd_{h}_{qb}",
                                   tag="rstd")
                    nc.vector.reciprocal_approx_fast(rstd[:], sds[h][:])
                    nc.vector.tensor_mul(ublk, ublk, rstd[:])
                for oc in range(NKC):
                    py = pyp.tile([128, 512], f32, name=f"py_{oc}_{qb}",
                                  tag="py")
                    for h2 in range(HPC):
                        nc.tensor.matmul(
                            py[:],
                            WO[h2][:, oc * 128:(oc + 1) * 128],
                            U[h2][:, qb * 512:(qb + 1) * 512],
                            start=(h2 == 0), stop=(h2 == HPC - 1))
                    yst = ys.tile([128, 512], bf16, name=f"yst_{oc}_{qb}",
                                  tag="yst")
                    if oc % 2 == 0:
                        nc.vector.tensor_copy(yst[:], py[:])
                    else:
                        nc.scalar.copy(yst[:], py[:])
                    nc.sync.dma_start(
                        out=y_t[oc * 128:(oc + 1) * 128,
                                qb * 512:(qb + 1) * 512],
                        in_=yst[:])

            pending = None
            for qb in range(NB):
                for h in range(HPC):
                    hp, a = h // 2, h % 2
                    pv = pvp.tile([128, 1024], f32, name=f"pv_{h}_{qb}",
                                  tag="pv")
                    esum = esp.tile([128, 1024], bf16, name=f"es_{h}_{qb}",
                                    tag="esum")
                    nkc = 4 * qb + 4

                    def pv_esum(E, E3, kc, qoff):
                        # pv + denominator accumulation for chunk kc; emitted
                        # one iteration late so the PE never waits on the exp
                        first, last = (kc == 0), (kc == nkc - 1)
                        for g in (0, 1):
                            nc.tensor.matmul(
                                pv[:, g * 512 + qoff:g * 512 + 512],
                                VH[kc][:, h * 128:(h + 1) * 128],
                                E[:, g * 512 + qoff:g * 512 + 512],
                                start=first, stop=last)
                        # all adds on DVE: at bf16 2x rate the serial chain
                        # is cheap, and single-engine in-order execution
                        # avoids a cross-engine semaphore hop per link
                        eng = nc.vector
                        if kc == 0:
                            nc.vector.tensor_copy(esum[:], E[:])
                        elif kc == nkc - 1:
                            pass  # folded into the smred matmul below
                        elif qoff == 0:
                            eng.tensor_add(esum[:], esum[:], E[:])
                        else:
                            esum3 = esum[:].rearrange("p (g q) -> p g q", g=2)
                            esl = esum3[:, :, qoff:512]
                            eng.tensor_add(esl, esl, E3[:, :, qoff:512])

                    prev = None
                    for kc in range(nkc):
                        j = kc - 4 * qb  # >= 0 on the causal diagonal band
                        qoff = j * 128 if j >= 0 else 0
                        ps = sp.tile([128, 1024], f32, name=f"s_{h}_{qb}_{kc}",
                                     tag="s")
                        for g in (0, 1):
                            tq = 2 * g + hp
                            nc.tensor.matmul(
                                ps[:, g * 512 + qoff:g * 512 + 512],
                                KZ[tq][a][:, kc * 128:(kc + 1) * 128],
                                QT[tq][:, qb * 512 + qoff:(qb + 1) * 512],
                                start=True, stop=True)
                        E = ep.tile([128, 1024], bf16, name=f"e_{h}_{qb}_{kc}",
                                    tag="e")
                        E3 = E[:].rearrange("p (g q) -> p g q", g=2)
                        if qoff == 0:
                            nc.scalar.activation(E[:], ps[:], Act.Exp,
                                                 scale=0.125)
                        else:
                            ps3 = ps[:].rearrange("p (g q) -> p g q", g=2)
                            nc.scalar.activation(E3[:, :, qoff:512],
                                                 ps3[:, :, qoff:512],
                                                 Act.Exp, scale=0.125)
                        if j >= 0:
                            # causal mask on the diagonal 128-col block of
                            # both maps in one strided op; on GpSimd (idle) —
                            # masks are independent, the DVE keeps the
                            # serial esum chain
                            sl = E3[:, :, qoff:qoff + 128]
                            nc.gpsimd.tensor_mul(sl, sl, m01d3)
                        if prev is not None:
                            pv_esum(*prev)
                        prev = (E, E3, kc, qoff)
                    pv_esum(*prev)
                    E_last = prev[0]
                    # denominator partition-reduce on PE; the final key chunk
                    # (always j=3, cols [384:512) per map) accumulates via a
                    # second matmul gated only by its exp, like pv
                    sm = sp.tile([128, 1024], f32, name=f"sm_{h}_{qb}", tag="s")
                    for g in (0, 1):
                        nc.tensor.matmul(
                            sm[:, g * 512:(g + 1) * 512], ones16[:],
                            esum[:, g * 512:(g + 1) * 512],
                            start=True, stop=False)
                        nc.tensor.matmul(
                            sm[:, g * 512 + 384:(g + 1) * 512], ones16[:],
                            E_last[:, g * 512 + 384:g * 512 + 512],
                            start=False, stop=True)
                    rb = cb.tile([128, 1024], f32, name=f"rb_{h}_{qb}",
                                 tag="rb")
                    nc.vector.reciprocal_approx_fast(rb[:], sm[:])
                    t12 = cb.tile([128, 1024], f32, name=f"t12_{h}_{qb}",
                                  tag="t12")
                    nc.vector.tensor_mul(t12[:], pv[:], rb[:])
                    ublk = U[h][:, qb * 512:(qb + 1) * 512]
                    # U = (t2 * -lam) + t1
                    nc.vector.scalar_tensor_tensor(
                        ublk, t12[:, 512:1024], neglam[:], t12[:, 0:512],
                        op0=Alu.mult, op1=Alu.add)
                    if h == 0 and pending is not None:
                        rms_outproj(pending)
                        pending = None
                pending = qb
            rms_outproj(pending)

    nc.compile()
    return nc


def _host_prep(q, k, v, Wq, Wk, Wv, Wo, lambda_q1, lambda_k1, lambda_q2,
               lambda_k2, gnorm_w, cos_emb, sin_emb):
    import ml_dtypes

    f32 = np.float32
    bf16 = ml_dtypes.bfloat16
    q = np.asarray(q, f32); k = np.asarray(k, f32); v = np.asarray(v, f32)
    Wq = np.asarray(Wq, f32); Wk = np.asarray(Wk, f32)
    Wv = np.asarray(Wv, f32); Wo = np.asarray(Wo, f32)
    gnorm_w = np.asarray(gnorm_w, f32)
    cos_emb = np.asarray(cos_emb, f32); sin_emb = np.asarray(sin_emb, f32)

    lam1 = np.exp(np.sum(np.asarray(lambda_q1, f32) * np.asarray(lambda_k1, f32),
                         dtype=f32))
    lam2 = np.exp(np.sum(np.asarray(lambda_q2, f32) * np.asarray(lambda_k2, f32),
                         dtype=f32))
    lam = np.float32(lam1 - lam2 + LAMBDA_INIT)

    # per-batch transposed activations (bf16)
    xt = {}
    for b in range(B):
        xt[("q", b)] = np.ascontiguousarray(q[b].T).astype(bf16)
        xt[("k", b)] = np.ascontiguousarray(k[b].T).astype(bf16)
        xt[("v", b)] = np.ascontiguousarray(v[b].T).astype(bf16)

    # shared constant tensors
    base_c = cos_emb[:S, :QKD]          # [S, 64]
    base_s = sin_emb[:S, :QKD]
    crep = np.ascontiguousarray(np.tile(base_c.T, (2, 1))).astype(bf16)
    srep = np.ascontiguousarray(np.tile(base_s.T, (2, 1))).astype(bf16)
    pmat = np.zeros((128, 128), f32)
    for blk in range(2):
        o = blk * 64
        for i in range(QKD // 2):
            pmat[o + 2 * i, o + 2 * i + 1] = 1.0     # lhsT[2i, 2i+1]
            pmat[o + 2 * i + 1, o + 2 * i] = -1.0    # lhsT[2i+1, 2i]
    pmat = pmat.astype(bf16)
    # m01[p, c] = 0 where key p > query c (within diagonal block), else 1;
    # duplicated side by side so one strided op masks both softmax maps
    m01 = (np.arange(128)[:, None] <= np.arange(128)[None, :]).astype(f32)
    m01d = np.ascontiguousarray(np.concatenate([m01, m01], axis=1)).astype(bf16)
    ones16 = np.ones((128, 128), f32).astype(bf16)
    ones_a = np.ones((128, 128), f32)
    neglam = np.full((128, 1), -lam, f32)

    per_core = []
    for c in range(NCORES):
        b, grp = c // GRPS, c % GRPS
        heads = [HPC * grp + j for j in range(HPC)]
        # wq/wk columns: tile t = 2*g + hp; within tile: head 2*hp+a at
        # cols [64*a, 64*a+64), original feature order (interleaved pairs)
        cols = []
        for t in range(4):
            g, hp = t // 2, t % 2
            for a2 in range(2):
                hg = heads[2 * hp + a2]
                cols.extend(hg * HD + g * QKD + d for d in range(QKD))
        cols = np.asarray(cols)
        vrows = np.asarray([h * HD + d for h in heads for d in range(HD)])
        wq_t = np.ascontiguousarray(Wq[cols, :].T).astype(bf16)
        wk_t = np.ascontiguousarray(Wk[cols, :].T).astype(bf16)
        wv_t = np.ascontiguousarray(Wv[vrows, :].T).astype(bf16)
        gtile = np.tile(gnorm_w, HPC)                       # [512]
        wo_t = np.ascontiguousarray(
            ((1.0 - LAMBDA_INIT) * Wo[:, vrows] * gtile[None, :]).T
        ).astype(bf16)
        per_core.append({
            "xq_t": xt[("q", b)], "xk_t": xt[("k", b)], "xv_t": xt[("v", b)],
            "wq_t": wq_t, "wk_t": wk_t, "wv_t": wv_t, "wo_t": wo_t,
            "crep": crep, "srep": srep, "pmat": pmat,
            "m01d": m01d, "ones16": ones16,
            "ones_a": ones_a, "neglam": neglam,
        })
    return per_core


def _install_ntff_hook():
    """antenv.axon_hooks is absent in this image; synthesize it so
    run_bass_kernel_spmd(trace=True) can capture NTFF profiles."""
    import sys as _sys
    import types

    if "antenv.axon_hooks" in _sys.modules:
        return
    import antenv
    mod = types.ModuleType("antenv.axon_hooks")
    state = {"hook": None}
    mod.set_axon_ntff_profile_hook = lambda h: state.__setitem__("hook", h)
    mod.get_axon_ntff_profile_hook = lambda: state["hook"]
    _sys.modules["antenv.axon_hooks"] = mod
    antenv.axon_hooks = mod
    try:
        from trn_agent_boot.trn_boot import _ntff_profile_via_ctypes
        state["hook"] = _ntff_profile_via_ctypes("/opt/axon/libaxon_pjrt.so")
    except Exception as e:  # degrade: trace skipped, run still works
        print("ntff hook install failed:", e)


def kernel(q, k, v, Wq, Wk, Wv, Wo, lambda_q1, lambda_k1, lambda_q2,
           lambda_k2, gnorm_w, cos_emb, sin_emb, mask, _trace=False):
    if _trace:
        _install_ntff_hook()
    global _PROGRAM
    if _PROGRAM is None:
        _PROGRAM = _build_program()
    nc = _PROGRAM

    in_maps = _host_prep(q, k, v, Wq, Wk, Wv, Wo, lambda_q1, lambda_k1,
                         lambda_q2, lambda_k2, gnorm_w, cos_emb, sin_emb)

    from concourse.bass_utils import run_bass_kernel_spmd
    res = run_bass_kernel_spmd(nc, in_maps, core_ids=list(range(NCORES)),
                               trace=_trace)
    kernel.last_result = res

    y = np.zeros((B, S, HID), np.float32)
    for c in range(NCORES):
        y[c // GRPS] += res.results[c]["y_t"].T.astype(np.float32)
    return y
